# revision 5
# baseline (speedup 1.0000x reference)
"""GCN (4-layer, GCNConv + BN + residual + log_softmax) on 8 Trainium2 NeuronCores.

Strategy (graph/data parallel, dst-owner sharding):
  - nodes split 12500/core; edges owned by dst core; self-loops added as edges
  - symmetric norm dis[src]*dis[dst] factored: gather sources pre-scaled by
    dis (x' = dis*x on host, hw' = dis*(hW) on device), dst side applied via a
    replicated-row multiply (disrep)
  - per layer: hw' = dis*(h@W) -> 8-rank AllGather (bf16) into a shared DRAM
    buffer -> dma_gather of edge src rows (int16 idx, 4 source chunks of 25000
    rows) -> one-hot selector matmuls on TensorE (128-edge blocks, PSUM
    accumulated per dst tile) -> SBUF accumulation across chunks -> BN/relu/
    residual epilogue on ACT/DVE
  - layer 0 gathers directly from a host-shipped full x' (no collective);
    layer 3 aggregates h3' then applies W3 [128,2] + log_softmax

Self-contained: only needs numpy + the system concourse install.
"""
import sys
import numpy as np

sys.path.insert(0, "/opt/trn_rl_repo")

import ml_dtypes

BF16 = ml_dtypes.bfloat16

BN_EPS = 1e-5


class Cfg:
    def __init__(self, N, E, ncores, chunk, gb=32):
        self.N = N
        self.E = E
        self.NCORES = ncores
        self.NPC = N // ncores              # nodes per core
        self.NT = (self.NPC + 127) // 128   # dst tiles per core
        self.AGGW = self.NT * 128           # padded agg width
        self.NCHUNK = (N + chunk - 1) // chunk
        self.CHUNK = chunk                  # gather source chunk rows (< 32768)
        self.GB = gb                        # max blocks per dma_gather op


FULL = Cfg(N=100000, E=800000, ncores=8, chunk=25000)


def _prep(cfg, x, edge_index, rng_pad_row=0):
    """Host-side graph prep. Returns (meta, per_core_inputs)."""
    N, NPC, NT, NCHUNK, CHUNK = cfg.N, cfg.NPC, cfg.NT, cfg.NCHUNK, cfg.CHUNK
    NC = cfg.NCORES
    src = np.asarray(edge_index[0], np.int64)
    dst = np.asarray(edge_index[1], np.int64)
    deg = np.bincount(dst, minlength=N).astype(np.float32) + 1.0
    dis = 1.0 / np.sqrt(deg)

    s_all = np.concatenate([src, np.arange(N, dtype=np.int64)])
    d_all = np.concatenate([dst, np.arange(N, dtype=np.int64)])
    core = d_all // NPC
    lo = d_all - core * NPC
    tile = lo >> 7
    chunk = s_all // CHUNK

    key = (core * NCHUNK + chunk) * NT + tile
    order = np.argsort(key, kind="stable")
    ks = key[order]
    ss = s_all[order]
    ll = lo[order]

    counts = np.bincount(key, minlength=NC * NCHUNK * NT).reshape(NC, NCHUNK, NT)
    blocks = (counts + 127) // 128
    blocks = blocks.max(axis=0)           # [NCHUNK, NT] shared SPMD structure
    blocks[0] = np.maximum(blocks[0], 1)  # chunk-0 group initializes agg slice
    NBLK = int(blocks.sum())
    S = NBLK * 128
    slot_off = np.zeros((NCHUNK, NT), np.int64)
    slot_off.ravel()[1:] = np.cumsum(blocks.ravel())[:-1]
    slot_off *= 128

    # rank of each edge within its (core, chunk, tile) group
    grp_start = np.zeros(NC * NCHUNK * NT, np.int64)
    np.cumsum(counts.ravel()[:-1], out=grp_start[1:])
    rank = np.arange(len(ks)) - grp_start[ks]
    cc = ks // (NCHUNK * NT)
    rem = ks - cc * (NCHUNK * NT)
    ch = rem // NT
    tt = rem - ch * NT
    slot = slot_off[ch, tt] + rank

    gidx = np.full((NC, S), rng_pad_row, np.int64)   # pad slots gather row 0
    dstc = np.full((NC, S), -1.0, np.float32)        # pad slots: no dst match
    gidx[cc, slot] = ss - ch * CHUNK
    dstc[cc, slot] = (ll - tt * 128).astype(np.float32)

    assert gidx.max() < 32768 and gidx.min() >= 0

    # wrap idx: position i -> [i % 16, i // 16], replicated across 8 groups
    gw = np.zeros((NC, 128, S // 16), np.int16)
    resh = gidx.reshape(NC, S // 16, 16).astype(np.int16)
    for g in range(8):
        gw[:, g * 16:(g + 1) * 16, :] = resh.transpose(0, 2, 1)
    # dst columns: [128, NBLK] col b partition p = dstc[b*128+p]
    dcols = dstc.reshape(NC, NBLK, 128).transpose(0, 2, 1).astype(BF16)

    # per-core dis layouts
    dl = np.ones((NC, 128, NT), np.float32)          # dl[c][p,t]=dis[c*NPC+t*128+p]
    disrep = np.zeros((NC, 128, cfg.AGGW), np.float32)
    for c in range(NC):
        dv = dis[c * NPC:(c + 1) * NPC]
        pad = np.zeros(cfg.AGGW - NPC, np.float32)
        dvp = np.concatenate([dv, pad])
        dl[c] = np.concatenate([dv, np.ones(cfg.AGGW - NPC, np.float32)]
                               ).reshape(NT, 128).T
        disrep[c] = np.broadcast_to(dvp[None, :], (128, cfg.AGGW))

    xg = (np.asarray(x, np.float32) * dis[:, None]).astype(BF16)  # full, pre-scaled

    meta = dict(blocks=blocks, NBLK=NBLK, S=S)
    per_core = []
    for c in range(NC):
        per_core.append(dict(
            gidx=gw[c],
            dstc=dcols[c],
            dl=dl[c],
            disrep=disrep[c].astype(BF16),
            xg=xg,
        ))
    return meta, per_core, dis


def build_nc(cfg, meta):
    import concourse.tile as tile
    import concourse.mybir as mybir
    from concourse.bacc import Bacc

    N, NPC, NT, NCHUNK, CHUNK = cfg.N, cfg.NPC, cfg.NT, cfg.NCHUNK, cfg.CHUNK
    AGGW, GB = cfg.AGGW, cfg.GB
    blocks, NBLK, S = meta["blocks"], meta["NBLK"], meta["S"]
    f32, bf16, i16 = mybir.dt.float32, mybir.dt.bfloat16, mybir.dt.int16
    AF = mybir.ActivationFunctionType
    OP = mybir.AluOpType

    nc = Bacc("TRN2", num_devices=cfg.NCORES)
    P = 128

    xg = nc.declare_dram_parameter("xg", [N, 128], bf16, isOutput=False)
    gidx = nc.declare_dram_parameter("gidx", [P, S // 16], i16, isOutput=False)
    dstc_d = nc.declare_dram_parameter("dstc", [P, NBLK], bf16, isOutput=False)
    dl_d = nc.declare_dram_parameter("dl", [P, NT], f32, isOutput=False)
    disrep_d = nc.declare_dram_parameter("disrep", [P, AGGW], bf16, isOutput=False)
    Wd = [nc.declare_dram_parameter(f"W{i}", [128, 128 if i < 3 else 2], bf16,
                                    isOutput=False) for i in range(4)]
    b0_d = nc.declare_dram_parameter("b0", [P, 1], f32, isOutput=False)
    s0_d = nc.declare_dram_parameter("s0", [P, 1], f32, isOutput=False)
    sh0_d = nc.declare_dram_parameter("sh0", [P, 1], f32, isOutput=False)
    s1_d = nc.declare_dram_parameter("s1", [P, 1], f32, isOutput=False)
    sh1_d = nc.declare_dram_parameter("sh1", [P, 1], f32, isOutput=False)
    b3_d = nc.declare_dram_parameter("b3", [P, 2], f32, isOutput=False)
    out_d = nc.declare_dram_parameter("out", [NPC, 2], f32, isOutput=True)

    cc_in = [nc.dram_tensor(f"cc_in{l}", [NPC, 128], bf16, kind="Internal")
             for l in (1, 2, 3)]
    hw_full = [nc.dram_tensor(f"hw_full{l}", [N, 128], bf16, kind="Internal",
                              addr_space="Shared") for l in (1, 2, 3)]

    groups = [list(range(cfg.NCORES))]

    with tile.TileContext(nc) as tc:
        with tc.tile_pool(name="cpool", bufs=1) as cpool, \
             tc.tile_pool(name="gpool", bufs=2) as gpool, \
             tc.tile_pool(name="spool", bufs=4) as spool, \
             tc.tile_pool(name="dpool", bufs=2) as dpool, \
             tc.tile_pool(name="hwpool", bufs=3) as hwpool, \
             tc.tile_pool(name="apsum", bufs=4, space="PSUM") as apsum, \
             tc.tile_pool(name="wpsum", bufs=2, space="PSUM") as wpsum:

            # ---- persistent state + constants ----
            hA = cpool.tile([P, AGGW], f32, tag="hA")
            hB = cpool.tile([P, AGGW], f32, tag="hB")
            agg = cpool.tile([P, AGGW], f32, tag="agg")
            hTb = cpool.tile([P, AGGW], bf16, tag="hTb")
            dstc_t = cpool.tile([P, NBLK], bf16, tag="dstc")
            nc.sync.dma_start(out=dstc_t[:], in_=dstc_d[:])
            dl_t = cpool.tile([P, NT], f32, tag="dl")
            nc.sync.dma_start(out=dl_t[:], in_=dl_d[:])
            iota = cpool.tile([P, P], bf16, tag="iota")
            nc.gpsimd.iota(iota[:], pattern=[[1, P]], base=0, channel_multiplier=0,
                           allow_small_or_imprecise_dtypes=True)
            Wt = []
            for i in range(4):
                w = cpool.tile([128, 128 if i < 3 else 2], bf16, tag=f"W{i}")
                nc.sync.dma_start(out=w[:], in_=Wd[i][:])
                Wt.append(w)
            scal = {}
            for nm, d in (("b0", b0_d), ("s0", s0_d), ("sh0", sh0_d),
                          ("s1", s1_d), ("sh1", sh1_d)):
                t = cpool.tile([P, 1], f32, tag=nm)
                nc.sync.dma_start(out=t[:], in_=d[:])
                scal[nm] = t
            b3_t = cpool.tile([P, 2], f32, tag="b3")
            nc.sync.dma_start(out=b3_t[:], in_=b3_d[:])

            # chunk slot offsets (blocks)
            chunk_blk = blocks.sum(axis=1)          # blocks per chunk
            chunk_off = np.zeros(NCHUNK + 1, np.int64)
            chunk_off[1:] = np.cumsum(chunk_blk)

            def aggregate(src_dram):
                """agg[:, t*128:(t+1)*128] = sum over edges of src rows (bf16
                gather) selected into dst one-hot columns; SBUF accumulation
                across chunks."""
                bi = 0  # global block index
                for c in range(NCHUNK):
                    nb_c = int(chunk_blk[c])
                    # gather windows for this chunk
                    win = []  # (tile handle, first block)
                    wstart = 0
                    while wstart < nb_c:
                        wn = min(GB, nb_c - wstart)
                        g = gpool.tile([P, GB, 128], bf16, tag="gath")
                        it = dpool.tile([P, (GB * 128) // 16], i16, tag="gix")
                        s0 = (chunk_off[c] + wstart) * 128
                        nc.sync.dma_start(
                            out=it[:, : (wn * 128) // 16],
                            in_=gidx[:, s0 // 16: (s0 + wn * 128) // 16])
                        nc.gpsimd.dma_gather(
                            out_ap=g[:, :wn, :],
                            in_ap=src_dram[c * CHUNK:, :],
                            idxs_ap=it[:, : (wn * 128) // 16],
                            num_idxs=wn * 128,
                            num_idxs_reg=wn * 128,
                            elem_size=128,
                            single_packet=False,
                        )
                        win.append((g, wstart, wn))
                        wstart += wn
                    wi = 0
                    wpos = 0
                    for t in range(NT):
                        nb = int(blocks[c][t])
                        if nb == 0:
                            continue
                        ps = apsum.tile([P, P], f32, space="PSUM", tag="ps")
                        for k in range(nb):
                            g, wfirst, wn = win[wi]
                            j = wpos
                            sel = spool.tile([P, P], bf16, tag="sel")
                            nc.vector.tensor_tensor(
                                out=sel[:],
                                in0=dstc_t[:, bi:bi + 1].to_broadcast([P, P]),
                                in1=iota[:],
                                op=OP.is_equal)
                            nc.tensor.matmul(out=ps[:], lhsT=g[:, j, :], rhs=sel[:],
                                             start=(k == 0), stop=(k == nb - 1))
                            bi += 1
                            wpos += 1
                            if wpos == wn:
                                wi += 1
                                wpos = 0
                        sl = agg[:, t * P:(t + 1) * P]
                        if c == 0:
                            nc.vector.tensor_copy(out=sl, in_=ps[:])
                        else:
                            nc.vector.tensor_tensor(out=sl, in0=sl, in1=ps[:],
                                                    op=OP.add)
                assert bi == NBLK

            def scale_disrep(out_tile, out_dtype_note=None):
                """out_tile[:, w] = agg[:, w] * disrep[:, w] (25 wide ops)."""
                for w0 in range(0, AGGW, 512):
                    w1 = min(w0 + 512, AGGW)
                    dr = dpool.tile([P, 512], bf16, tag="drs")
                    nc.sync.dma_start(out=dr[:, :w1 - w0], in_=disrep_d[:, w0:w1])
                    nc.vector.tensor_tensor(
                        out=out_tile[:, w0:w1], in0=agg[:, w0:w1],
                        in1=dr[:, :w1 - w0], op=OP.mult)

            def hw_chain(h_bf16, W, cc_dram):
                """cc_dram rows = dis * (h @ W) in bf16, per 128-node tile."""
                for t in range(NT):
                    ps = apsum.tile([P, P], f32, space="PSUM", tag="ps")
                    nc.tensor.matmul(out=ps[:], lhsT=h_bf16[:, t * P:(t + 1) * P],
                                     rhs=W[:], start=True, stop=True)
                    hw = hwpool.tile([P, P], bf16, tag="hwt")
                    nc.vector.tensor_tensor(
                        out=hw[:], in0=ps[:],
                        in1=dl_t[:, t:t + 1].to_broadcast([P, P]), op=OP.mult)
                    r0 = t * P
                    r1 = min(r0 + P, NPC)
                    nc.sync.dma_start(out=cc_dram[r0:r1, :], in_=hw[:r1 - r0, :])

            def allgather(l):
                nc.gpsimd.collective_compute(
                    "AllGather", mybir.AluOpType.bypass,
                    replica_groups=groups,
                    ins=[cc_in[l][:]], outs=[hw_full[l][:]])

            # ================= layer 0 =================
            aggregate(xg)
            scale_disrep(hTb)                       # aggs0 (bf16) in hTb slot
            for w0 in range(0, AGGW, 512):
                w1 = min(w0 + 512, AGGW)
                ps = wpsum.tile([P, 512], f32, space="PSUM", tag="wps")
                nc.tensor.matmul(out=ps[:, :w1 - w0], lhsT=Wt[0][:],
                                 rhs=hTb[:, w0:w1], start=True, stop=True)
                nc.scalar.activation(out=hA[:, w0:w1], in_=ps[:, :w1 - w0],
                                     func=AF.Relu, bias=scal["b0"][:, :1], scale=1.0)
            for w0 in range(0, AGGW, 512):          # bf16 copy for matmul lhsT
                w1 = min(w0 + 512, AGGW)
                nc.vector.tensor_copy(out=hTb[:, w0:w1], in_=hA[:, w0:w1])
            hw_chain(hTb, Wt[1], cc_in[0])
            allgather(0)

            # ================= layer 1 =================
            aggregate(hw_full[0])
            scale_disrep(agg)                       # in-place
            for w0 in range(0, AGGW, 512):
                w1 = min(w0 + 512, AGGW)
                tmp = hwpool.tile([P, 512], f32, tag="rl")
                nc.scalar.activation(out=tmp[:, :w1 - w0], in_=agg[:, w0:w1],
                                     func=AF.Relu, bias=scal["sh0"][:, :1],
                                     scale=scal["s0"][:, :1])
                nc.vector.tensor_tensor(out=hB[:, w0:w1], in0=tmp[:, :w1 - w0],
                                        in1=hA[:, w0:w1], op=OP.add)
            for w0 in range(0, AGGW, 512):
                w1 = min(w0 + 512, AGGW)
                nc.vector.tensor_copy(out=hTb[:, w0:w1], in_=hB[:, w0:w1])
            hw_chain(hTb, Wt[2], cc_in[1])
            allgather(1)

            # ================= layer 2 =================
            aggregate(hw_full[1])
            scale_disrep(agg)
            for w0 in range(0, AGGW, 512):
                w1 = min(w0 + 512, AGGW)
                tmp = hwpool.tile([P, 512], f32, tag="rl")
                nc.scalar.activation(out=tmp[:, :w1 - w0], in_=agg[:, w0:w1],
                                     func=AF.Relu, bias=scal["sh1"][:, :1],
                                     scale=scal["s1"][:, :1])
                nc.vector.tensor_tensor(out=hA[:, w0:w1], in0=tmp[:, :w1 - w0],
                                        in1=hB[:, w0:w1], op=OP.add)
            for w0 in range(0, AGGW, 512):
                w1 = min(w0 + 512, AGGW)
                nc.vector.tensor_copy(out=hTb[:, w0:w1], in_=hA[:, w0:w1])
            # h3' node-major = dis * h3 via PE transpose of hTb tiles
            from concourse.masks import make_identity
            ident = cpool.tile([P, P], bf16, tag="ident")
            make_identity(nc, ident[:])
            for t in range(NT):
                ps = apsum.tile([P, P], bf16, space="PSUM", tag="ps")
                nc.tensor.transpose(out=ps[:], in_=hTb[:, t * P:(t + 1) * P],
                                    identity=ident[:])
                hw = hwpool.tile([P, P], bf16, tag="hwt")
                nc.vector.tensor_tensor(
                    out=hw[:], in0=ps[:],
                    in1=dl_t[:, t:t + 1].to_broadcast([P, P]), op=OP.mult)
                r0, r1 = t * P, min(t * P + P, NPC)
                nc.sync.dma_start(out=cc_in[2][r0:r1, :], in_=hw[:r1 - r0, :])
            allgather(2)

            # ================= layer 3 =================
            aggregate(hw_full[2])
            scale_disrep(hTb)                        # aggs3 bf16
            wide = cpool.tile([P, 2, NT], f32, tag="wide")
            ps3 = cpool.tile([P, 2, NT], f32, tag="ps3")
            for t in range(NT):
                ps = apsum.tile([P, 2], f32, space="PSUM", tag="ps")
                nc.tensor.matmul(out=ps[:], lhsT=hTb[:, t * P:(t + 1) * P],
                                 rhs=Wt[3][:], start=True, stop=True)
                nc.vector.tensor_tensor(out=wide[:, :, t:t + 1], in0=ps[:, :],
                                        in1=b3_t[:, :2], op=OP.add)
            c0 = wide[:, 0, :]
            c1 = wide[:, 1, :]
            mx = cpool.tile([P, NT], f32, tag="mx")
            nc.vector.tensor_tensor(out=mx[:], in0=c0, in1=c1, op=OP.max)
            tt0 = cpool.tile([P, NT], f32, tag="tt0")
            tt1 = cpool.tile([P, NT], f32, tag="tt1")
            nc.vector.tensor_tensor(out=tt0[:], in0=c0, in1=mx[:], op=OP.subtract)
            nc.vector.tensor_tensor(out=tt1[:], in0=c1, in1=mx[:], op=OP.subtract)
            e0 = cpool.tile([P, NT], f32, tag="e0")
            e1 = cpool.tile([P, NT], f32, tag="e1")
            nc.scalar.activation(out=e0[:], in_=tt0[:], func=AF.Exp)
            nc.scalar.activation(out=e1[:], in_=tt1[:], func=AF.Exp)
            sm = cpool.tile([P, NT], f32, tag="sm")
            nc.vector.tensor_tensor(out=sm[:], in0=e0[:], in1=e1[:], op=OP.add)
            ls = cpool.tile([P, NT], f32, tag="ls")
            nc.scalar.activation(out=ls[:], in_=sm[:], func=AF.Ln)
            nc.vector.tensor_tensor(out=ps3[:, 0, :], in0=tt0[:], in1=ls[:],
                                    op=OP.subtract)
            nc.vector.tensor_tensor(out=ps3[:, 1, :], in0=tt1[:], in1=ls[:],
                                    op=OP.subtract)
            for t in range(NT):
                r0, r1 = t * P, min(t * P + P, NPC)
                nc.sync.dma_start(out=out_d[r0:r1, :], in_=ps3[:r1 - r0, :, t])

    nc.finalize()
    return nc


_BUILT = {}


def _get_built(cfg, meta):
    key = (cfg.N, cfg.E, meta["NBLK"])
    if key not in _BUILT:
        _BUILT[key] = build_nc(cfg, meta)
    return _BUILT[key]


def kernel(x, edge_index, W0, b0, W1, b1, W2, b2, W3, b3,
           bn0_g, bn0_b, bn0_m, bn0_v, bn1_g, bn1_b, bn1_m, bn1_v,
           cfg=FULL, _run_sim=False):
    from concourse.bass_utils import run_bass_kernel_spmd

    import time as _time
    _t0 = _time.time()
    meta, per_core, dis = _prep(cfg, x, edge_index)
    print(f"[kernel] prep {_time.time()-_t0:.1f}s NBLK={meta['NBLK']} S={meta['S']}",
          file=sys.stderr)
    _t1 = _time.time()
    nc = _get_built(cfg, meta)
    print(f"[kernel] build {_time.time()-_t1:.1f}s", file=sys.stderr)

    s0 = (np.asarray(bn0_g, np.float32) /
          np.sqrt(np.asarray(bn0_v, np.float32) + BN_EPS))
    sh0 = (np.asarray(b1, np.float32) - np.asarray(bn0_m, np.float32)) * s0 \
        + np.asarray(bn0_b, np.float32)
    s1 = (np.asarray(bn1_g, np.float32) /
          np.sqrt(np.asarray(bn1_v, np.float32) + BN_EPS))
    sh1 = (np.asarray(b2, np.float32) - np.asarray(bn1_m, np.float32)) * s1 \
        + np.asarray(bn1_b, np.float32)

    common = dict(
        W0=np.asarray(W0, np.float32).astype(BF16),
        W1=np.asarray(W1, np.float32).astype(BF16),
        W2=np.asarray(W2, np.float32).astype(BF16),
        W3=np.asarray(W3, np.float32).astype(BF16),
        b0=np.asarray(b0, np.float32).reshape(128, 1),
        s0=s0.reshape(128, 1), sh0=sh0.reshape(128, 1),
        s1=s1.reshape(128, 1), sh1=sh1.reshape(128, 1),
        b3=np.broadcast_to(np.asarray(b3, np.float32)[None, :], (128, 2)).copy(),
    )
    in_maps = [{**pc, **common} for pc in per_core]

    if _run_sim:
        from concourse.bass_interp import MultiCoreSim
        sim = MultiCoreSim(nc, num_cores=cfg.NCORES)
        for c in range(cfg.NCORES):
            for k, v in in_maps[c].items():
                sim.cores[c].tensor(k)[:] = v
        sim.simulate()
        outs = [np.array(sim.cores[c].tensor("out")) for c in range(cfg.NCORES)]
    else:
        _t2 = _time.time()
        res = run_bass_kernel_spmd(nc, in_maps, core_ids=list(range(cfg.NCORES)))
        print(f"[kernel] run {_time.time()-_t2:.1f}s exec_ns={res.exec_time_ns}",
              file=sys.stderr)
        outs = [res.results[c]["out"] for c in range(cfg.NCORES)]

    return np.concatenate(outs, axis=0)[:cfg.N].astype(np.float32)
